# revision 7
# baseline (speedup 1.0000x reference)
"""DML (Chamfer-style) L1 loss kernel for Trainium2, 8 NeuronCores.

Math: for each batch b:
  pred2gt_min[j] = min_i ||pred[b,i] - gt[b,j]||_1       (queries = gt)
  gt2pred_min[j] = min_i ||gt_valid[b,i] - pred[b,j]||_1 (queries = pred)
  out = (mean(pred2gt_min) + mean(gt2pred_min)) / 2

Device mapping (data-parallel over B across 8 cores; 4 batches x 2 sides
= 8 "batch-sides" per core). Rotate coords 45 deg on host (u = x+y,
v = x-y) so L1 dist = max(|du|, |dv|). Per 128-query x 2048-candidate tile:
  - TensorE: du[p, j] = u_c[j] - u_q[p] via a K=6 bf16 ones-matmul:
      u split 3-way into bf16 parts (u = a0+a1+a2 with ~2^-27 residual);
      lhsT rows [-a0,-a1,-a2, 1,1,1], rhs rows [1,1,1, b0,b1,b2].
      All products have a 1.0 factor so they are exact; PSUM accumulates
      in fp32 -> du exact to ~5e-8. bf16 matmul streams 1 column/cycle
      (fp32 would be 4x slower).
  - ScalarE: |dv| = Abs(v_c_rep - v_q[p]) via Abs activation with
      per-partition bias (v_c replicated across partitions by DMA).
  - VectorE: one fused custom-DVE op per tile:
      out = max(max(du, -du), |dv|);  accum_out = min-reduce(out)
      (registered per-NEFF via the custom DVE table mechanism).
  - Host: means in float64, final scalar.
"""
import os
import numpy as np

import concourse.bacc as bacc
import concourse.mybir as mybir
import concourse.tile as tile
from concourse.bass_utils import run_bass_kernel_spmd

F32 = mybir.dt.float32
BF16 = mybir.dt.bfloat16
B, PNUM, D = 32, 2048, 2
NCORES = 8
BPC = B // NCORES          # batches per core
NSIDES = 2 * BPC           # batch-sides per core
P = 128                    # SBUF partitions
NCH = PNUM // P            # query chunks per batch-side
KS = 6                     # matmul contraction: 3 bf16 splits x 2 operands
BIG = 3.0e38

_CACHED = {}


def _register_fused_op():
    """Per-NEFF custom DVE op:
        out = max(max(in0, -in0), in1);  accum_out = min(s0, min(out))
    i.e. a fused |du| + max + min-reduce (the TensorTensorReduce the
    stock ucode lacks, with the abs folded in).
    """
    import concourse.dve_ops as dve_ops
    name = "MIN_OF_ABSMAX_ANT"
    if "fused_op" in _CACHED:
        return _CACHED["fused_op"]
    for o in dve_ops.OPS:
        if o.name == name:
            _CACHED["fused_op"] = o
            return o
    from concourse.dve_spec import Spec, Src0, Src1, C0, Zero, maxx, minn, lower
    from concourse.dve_uop import DveOpSpec

    spec = Spec(body=maxx(maxx(Src0, Zero - Src0), Src1), accum=minn, accum_init=C0)
    row = max(dve_ops._SUB_OPCODE_FOR_NAME.values()) + 1
    assert row < 0x20, "no free custom-DVE opcode rows"
    dve_ops._SUB_OPCODE_FOR_NAME[name] = row
    shas = {}
    for ver in ("v3", "v4"):
        tmp = DveOpSpec(name=name, opcode=row, uops=lower(spec, ver=ver), rd1_en=True)
        shas[ver] = tmp.sha(ver)
    op = dve_ops.DveOp(name, spec, subdim=False, uops_sha=shas)
    dve_ops.OPS.append(op)
    dve_ops.CUSTOM_DVE_SPECS[name] = spec
    _CACHED["fused_op"] = op
    return op


def _register_segmin_op():
    """Hand-edited custom DVE op SEGMIN_ABSMAX1_ANT:
        streams in0 (du, PSUM) and in1 (|dv|, SBUF — pre-abs'ed by ACT;
        the DVE reads at most one PSUM stream) with 3D [P, S, N] APs;
        value = min over j<=k of max(|in0[p,s,j]|, in1[p,s,j]), with the
        running min RESET at each page (subdim) boundary.
    Lower a plain global-scan spec, then add a SUB_DIM_DONE step state
    that reseeds the scan feedback flop from CONST_0 (s0=BIG) while
    consuming the first element of the new page — a segmented min-reduce,
    one instruction per S chunks.  `out` is a stride-0-inner broadcast AP
    over the [P, S] mins tile: the hardware writes the running min every
    element to the same per-page address, so the LAST write (= the page
    minimum) survives — no separate extraction pass."""
    import copy
    import concourse.dve_ops as dve_ops
    from concourse.dve_spec import Spec, Src0, Src1, C0, Zero, maxx, lower, scan, AluOp
    from concourse.dve_uop import DveOpSpec, AluInp, Trigger

    name = "SEGMIN_ABSMAX1_ANT"
    if "segop" in _CACHED:
        return _CACHED["segop"]
    for o in dve_ops.OPS:
        if o.name == name:
            _CACHED["segop"] = o
            return o

    e = maxx(maxx(Src0, Zero - Src0), Src1)
    spec = Spec(body=scan(AluOp.MIN, e, init=C0))
    row = max(dve_ops._SUB_OPCODE_FOR_NAME.values()) + 1
    assert row < 0x20, "no free custom-DVE opcode rows"
    dve_ops._SUB_OPCODE_FOR_NAME[name] = row

    shas = {}
    for ver in ("v3", "v4"):
        uops = lower(spec, ver=ver)
        assert len(uops) == 2, f"expected [seed, steady], got {len(uops)}"
        seed, steady = uops
        scan_blk = None
        for bi, blk in enumerate(steady.datapath_config):
            if blk.op == AluOp.MIN and (
                blk.alu_src0 == AluInp.CURR_ALU_OUT
                or blk.alu_src1 == AluInp.CURR_ALU_OUT
            ):
                scan_blk = bi
                break
        assert scan_blk is not None, "scan combine block not found"
        const_inp = seed.datapath_config[scan_blk].alu_src0
        assert AluInp.PREV_DELAY_0 <= const_inp <= AluInp.PREV_DELAY_0 + 5
        steady.trigger = (Trigger.SRC_TENSOR_DONE, Trigger.SUB_DIM_DONE, Trigger.NONE)
        steady.next_uop = (0, 2, 0)
        step = copy.deepcopy(steady)
        step.trigger = (Trigger.SRC_TENSOR_DONE, Trigger.SUB_DIM_DONE, Trigger.COUNT)
        step.next_uop = (0, 2, 1)
        step.repeat_count = 1
        blk = step.datapath_config[scan_blk]
        if blk.alu_src0 == AluInp.CURR_ALU_OUT:
            blk.alu_src0 = const_inp
        else:
            blk.alu_src1 = const_inp
        edited = DveOpSpec(name=name, opcode=row, uops=[seed, steady, step],
                           rd1_en=True)
        edited.validate(ver)
        shas[ver] = edited.sha(ver)
        dve_ops._COMPILE_CACHE[(name, ver)] = edited

    op = dve_ops.DveOp(name, spec, subdim=True, uops_sha=shas)
    dve_ops.OPS.append(op)
    dve_ops.CUSTOM_DVE_SPECS[name] = spec
    _CACHED["segop"] = op
    return op


def _build_seg(nwin: int, cpb: int = 4, repeat: int = 1):
    """Segmented-scan kernel: per cpb-chunk block, cpb K=12 matmuls fill one
    PSUM tile (one 512-f32 bank slot per chunk: du at [0,nwin), dv at
    [nwin,2nwin)); ONE batched ACT Abs moves dv→SBUF (the DVE reads at most
    one PSUM stream); ONE segmented-min DVE instruction reduces all cpb
    chunks, writing the page minima straight into the mins tile via a
    stride-0 out AP.  Per-instruction fixed costs amortize over cpb chunks."""
    assert NCH % cpb == 0 and 2 * nwin <= 512
    nc = bacc.Bacc("TRN2", target_bir_lowering=False)
    rhs_cols = NCH * 2 * nwin
    pemat = nc.dram_tensor(
        "pemat", [NSIDES, 12, PNUM + rhs_cols], BF16, kind="ExternalInput"
    )
    outm = nc.dram_tensor("mins", [NSIDES, P, NCH], F32, kind="ExternalOutput")
    segop = _register_segmin_op()

    with tile.TileContext(nc) as tc:
        with (
            tc.tile_pool(name="inp", bufs=2) as inp,
            tc.tile_pool(name="work", bufs=3) as work,
            tc.tile_pool(name="outp", bufs=2) as outp,
            tc.tile_pool(name="ps", bufs=2, space="PSUM") as ps,
        ):
            for rep in range(repeat):
                for s in range(NSIDES):
                    pm = inp.tile([12, PNUM + rhs_cols], BF16, tag="pm")
                    nc.gpsimd.dma_start(out=pm[:], in_=pemat[s])
                    mq = outp.tile([P, NCH], F32, tag="mq")
                    for b in range(NCH // cpb):
                        duv = ps.tile([P, cpb, 512], F32, tag="duv")
                        for j in range(cpb):
                            c = b * cpb + j
                            nc.tensor.matmul(
                                duv[:, j, 0:2 * nwin],
                                pm[:, c * P:(c + 1) * P],
                                pm[:, PNUM + c * 2 * nwin:PNUM + (c + 1) * 2 * nwin],
                                start=True,
                                stop=True,
                            )
                        dva = work.tile([P, cpb, nwin], F32, tag="dva")
                        nc.scalar.activation(
                            out=dva[:],
                            in_=duv[:, :, nwin:2 * nwin],
                            func=mybir.ActivationFunctionType.Abs,
                            bias=0.0,
                            scale=1.0,
                        )
                        mq_bc = mq[:, b * cpb:(b + 1) * cpb].rearrange(
                            "p (s n) -> p s n", n=1
                        ).broadcast_to([P, cpb, nwin])
                        nc.vector._custom_dve(
                            segop, out=mq_bc, in0=duv[:, :, 0:nwin],
                            in1=dva[:], s0=BIG,
                        )
                    nc.sync.dma_start(out=outm[s], in_=mq[:])
    nc.compile()
    return nc


def _build_win(nwin: int, repeat: int = 1):
    """Windowed kernel: queries and candidates sorted by u on host; query
    chunk c scans only the candidate-rank window [128c-W, 128c+127+W]
    (clipped; fixed width nwin). Exactness is certified on the host.

    Inputs per core:
      pemat [NSIDES, 6, PNUM + NCH*nwin] bf16 - lhsT query 3-splits then
            per-chunk candidate-u window 3-splits
      vwin  [NSIDES, 3, NCH*nwin] bf16 - per-chunk candidate-v 3-splits
            (broadcast to 128 partitions via a K=3 ones-matmul)
      vqneg [NSIDES, P, NCH] f32 - ACT bias (-v_q, sorted order)
    Output: mins [NSIDES, P, NCH] f32 (sorted query order).
    """
    nc = bacc.Bacc("TRN2", target_bir_lowering=False)
    rhs_cols = NCH * nwin
    pemat = nc.dram_tensor(
        "pemat", [NSIDES, KS, PNUM + rhs_cols], BF16, kind="ExternalInput"
    )
    vwin = nc.dram_tensor("vwin", [NSIDES, 3, rhs_cols], BF16, kind="ExternalInput")
    vqneg = nc.dram_tensor("vqneg", [NSIDES, P, NCH], F32, kind="ExternalInput")
    outm = nc.dram_tensor("mins", [NSIDES, P, NCH], F32, kind="ExternalOutput")
    fop = _register_fused_op()

    with tile.TileContext(nc) as tc:
        with (
            tc.tile_pool(name="ones", bufs=1) as onep,
            tc.tile_pool(name="inp", bufs=2) as inp,
            tc.tile_pool(name="work", bufs=6) as work,
            tc.tile_pool(name="outp", bufs=2) as outp,
            tc.tile_pool(name="ps", bufs=4, space="PSUM") as ps,
            tc.tile_pool(name="ps2", bufs=4, space="PSUM") as ps2,
        ):
            ones3 = onep.tile([3, P], BF16)
            nc.vector.memset(ones3[:], 1.0)
            for rep in range(repeat):
                for s in range(NSIDES):
                    pm = inp.tile([KS, PNUM + rhs_cols], BF16, tag="pm")
                    nc.gpsimd.dma_start(out=pm[:], in_=pemat[s])
                    vw = inp.tile([3, rhs_cols], BF16, tag="vw")
                    nc.gpsimd.dma_start(out=vw[:], in_=vwin[s])
                    vq = inp.tile([P, NCH], F32, tag="vq")
                    nc.gpsimd.dma_start(out=vq[:], in_=vqneg[s])
                    mq = outp.tile([P, NCH], F32, tag="mq")
                    for c in range(NCH):
                        du = ps.tile([P, nwin], F32, tag="du")
                        nc.tensor.matmul(
                            du[:],
                            pm[:, c * P:(c + 1) * P],
                            pm[:, PNUM + c * nwin:PNUM + (c + 1) * nwin],
                            start=True,
                            stop=True,
                        )
                        vbc = ps2.tile([P, nwin], F32, tag="vbc")
                        nc.tensor.matmul(
                            vbc[:],
                            ones3[:],
                            vw[:, c * nwin:(c + 1) * nwin],
                            start=True,
                            stop=True,
                        )
                        dva = work.tile([P, nwin], F32, tag="dva")
                        nc.scalar.activation(
                            out=dva[:],
                            in_=vbc[:],
                            func=mybir.ActivationFunctionType.Abs,
                            bias=vq[:, c:c + 1],
                            scale=1.0,
                        )
                        dmx = work.tile([P, nwin], F32, tag="dmx")
                        nc.vector._custom_dve(
                            fop, out=dmx[:], in0=du[:], in1=dva[:],
                            s0=BIG, accum_out=mq[:, c:c + 1],
                        )
                    nc.sync.dma_start(out=outm[s], in_=mq[:])
    nc.compile()
    return nc


def _build_win2(nwin: int, repeat: int = 1):
    """Like _build_win but one K=12 matmul per chunk computes both du and
    dv (signed) into one PSUM tile [P, 2*nwin]:
      lhsT rows: [-u_q splits(3), ones(3), -v_q splits(3), ones(3)]
      rhs du-cols: [1,1,1, b0,b1,b2, 0...]; dv-cols: [0..., 1,1,1, d0,d1,d2]
    ACT: |dv| = Abs(dv_psum) -> SBUF.  DVE: fused min-of-absmax.
    Inputs per core: pemat [NSIDES, 12, PNUM + NCH*2*nwin] bf16 only.
    """
    K12 = 12
    nc = bacc.Bacc("TRN2", target_bir_lowering=False)
    rhs_cols = NCH * 2 * nwin
    pemat = nc.dram_tensor(
        "pemat", [NSIDES, K12, PNUM + rhs_cols], BF16, kind="ExternalInput"
    )
    outm = nc.dram_tensor("mins", [NSIDES, P, NCH], F32, kind="ExternalOutput")
    fop = _register_fused_op()

    with tile.TileContext(nc) as tc:
        with (
            tc.tile_pool(name="inp", bufs=2) as inp,
            tc.tile_pool(name="work", bufs=6) as work,
            tc.tile_pool(name="outp", bufs=2) as outp,
            tc.tile_pool(name="ps", bufs=4, space="PSUM") as ps,
        ):
            for rep in range(repeat):
                for s in range(NSIDES):
                    pm = inp.tile([K12, PNUM + rhs_cols], BF16, tag="pm")
                    nc.gpsimd.dma_start(out=pm[:], in_=pemat[s])
                    mq = outp.tile([P, NCH], F32, tag="mq")
                    for c in range(NCH):
                        duv = ps.tile([P, 2 * nwin], F32, tag="duv")
                        nc.tensor.matmul(
                            duv[:],
                            pm[:, c * P:(c + 1) * P],
                            pm[:, PNUM + c * 2 * nwin:PNUM + (c + 1) * 2 * nwin],
                            start=True,
                            stop=True,
                        )
                        dva = work.tile([P, nwin], F32, tag="dva")
                        nc.scalar.activation(
                            out=dva[:],
                            in_=duv[:, nwin:2 * nwin],
                            func=mybir.ActivationFunctionType.Abs,
                            bias=0.0,
                            scale=1.0,
                        )
                        dmx = work.tile([P, nwin], F32, tag="dmx")
                        nc.vector._custom_dve(
                            fop, out=dmx[:], in0=duv[:, 0:nwin], in1=dva[:],
                            s0=BIG, accum_out=mq[:, c:c + 1],
                        )
                    nc.sync.dma_start(out=outm[s], in_=mq[:])
    nc.compile()
    return nc


def _build(repeat: int = 1):
    nc = bacc.Bacc("TRN2", target_bir_lowering=False)
    pemat = nc.dram_tensor("pemat", [NSIDES, KS, 2 * PNUM], BF16, kind="ExternalInput")
    vcand = nc.dram_tensor("vcand", [NSIDES, PNUM], F32, kind="ExternalInput")
    vqneg = nc.dram_tensor("vqneg", [NSIDES, P, NCH], F32, kind="ExternalInput")
    outm = nc.dram_tensor("mins", [NSIDES, P, NCH], F32, kind="ExternalOutput")
    fop = _register_fused_op()

    with tile.TileContext(nc) as tc:
        with (
            tc.tile_pool(name="inp", bufs=2) as inp,
            tc.tile_pool(name="work", bufs=3) as work,
            tc.tile_pool(name="outp", bufs=2) as outp,
            tc.tile_pool(name="ps", bufs=2, space="PSUM") as ps,
        ):
          for rep in range(repeat):
            for s in range(NSIDES):
                pm = inp.tile([KS, 2 * PNUM], BF16, tag="pm")
                nc.gpsimd.dma_start(out=pm[:], in_=pemat[s])
                vr = inp.tile([P, PNUM], F32, tag="vr")
                nc.gpsimd.dma_start(
                    out=vr[:], in_=vcand[s][None, :].broadcast_to([P, PNUM])
                )
                vq = inp.tile([P, NCH], F32, tag="vq")
                nc.gpsimd.dma_start(out=vq[:], in_=vqneg[s])
                mq = outp.tile([P, NCH], F32, tag="mq")
                for c in range(NCH):
                    du = ps.tile([P, PNUM], F32, tag="du")
                    for n in range(4):
                        nc.tensor.matmul(
                            du[:, n * 512:(n + 1) * 512],
                            pm[:, c * P:(c + 1) * P],
                            pm[:, PNUM + n * 512:PNUM + (n + 1) * 512],
                            start=True,
                            stop=True,
                        )
                    dva = work.tile([P, PNUM], F32, tag="dva")
                    nc.scalar.activation(
                        out=dva[:],
                        in_=vr[:],
                        func=mybir.ActivationFunctionType.Abs,
                        bias=vq[:, c:c + 1],
                        scale=1.0,
                    )
                    dmx = work.tile([P, PNUM], F32, tag="dmx")
                    nc.vector._custom_dve(
                        fop, out=dmx[:], in0=du[:], in1=dva[:],
                        s0=BIG, accum_out=mq[:, c:c + 1],
                    )
                nc.sync.dma_start(out=outm[s], in_=mq[:])
    nc.compile()
    return nc


def _mode():
    """(mode, nwin): mode 'seg' (default), 'win2', 'win', or 'full'."""
    m = os.environ.get("DML_MODE", "seg")
    nwin = int(os.environ.get("DML_NWIN", "192"))
    return m, nwin


def _get_nc(repeat: int = 1):
    m, nwin = _mode()
    key = ("nc", m, nwin, repeat)
    if key not in _CACHED:
        if m == "seg":
            cpb = int(os.environ.get("DML_CPB", "4"))
            _CACHED[key] = _build_seg(nwin, cpb, repeat)
        else:
            builder = {"win": _build_win, "win2": _build_win2}.get(m)
            _CACHED[key] = builder(nwin, repeat) if builder else _build(repeat)
    return _CACHED[key]


def _split3_bf16(x):
    """3-way bf16 split: x ~ s0+s1+s2 with ~2^-27 relative residual."""
    import ml_dtypes
    bf = ml_dtypes.bfloat16
    x = x.astype(np.float32)
    s0 = x.astype(bf)
    r1 = x - s0.astype(np.float32)
    s1 = r1.astype(bf)
    r2 = r1 - s1.astype(np.float32)
    s2 = r2.astype(bf)
    return s0, s1, s2


def _host_prep(pred, gt, gt_valid):
    import ml_dtypes
    bf = ml_dtypes.bfloat16
    pred = np.asarray(pred, dtype=np.float32)
    gt = np.asarray(gt, dtype=np.float32)
    gt_valid = np.asarray(gt_valid, dtype=np.float32)
    ones = np.ones(PNUM, bf)
    in_maps = []
    for core in range(NCORES):
        pemat = np.zeros((NSIDES, KS, 2 * PNUM), bf)
        vcand = np.empty((NSIDES, PNUM), np.float32)
        vqneg = np.empty((NSIDES, P, NCH), np.float32)
        for i in range(BPC):
            b = core * BPC + i
            for side in range(2):
                s = i * 2 + side
                if side == 0:   # pred2gt: candidates pred, queries gt
                    cand, query = pred[b], gt[b]
                else:           # gt2pred: candidates gt_valid, queries pred
                    cand, query = gt_valid[b], pred[b]
                u_c = cand[:, 0] + cand[:, 1]
                v_c = cand[:, 0] - cand[:, 1]
                u_q = query[:, 0] + query[:, 1]
                v_q = query[:, 0] - query[:, 1]
                a0, a1, a2 = _split3_bf16(-u_q)
                b0, b1, b2 = _split3_bf16(u_c)
                # lhsT half (queries): rows [-a0,-a1,-a2, 1,1,1]
                pemat[s, 0, :PNUM] = a0
                pemat[s, 1, :PNUM] = a1
                pemat[s, 2, :PNUM] = a2
                pemat[s, 3, :PNUM] = ones
                pemat[s, 4, :PNUM] = ones
                pemat[s, 5, :PNUM] = ones
                # rhs half (candidates): rows [1,1,1, b0,b1,b2]
                pemat[s, 0, PNUM:] = ones
                pemat[s, 1, PNUM:] = ones
                pemat[s, 2, PNUM:] = ones
                pemat[s, 3, PNUM:] = b0
                pemat[s, 4, PNUM:] = b1
                pemat[s, 5, PNUM:] = b2
                vcand[s] = v_c
                vqneg[s] = (-v_q).reshape(NCH, P).T
        in_maps.append({"pemat": pemat, "vcand": vcand, "vqneg": vqneg})
    return in_maps


def _host_prep_win(pred, gt, gt_valid, nwin):
    """Sorted-window prep. Returns (in_maps, certs) where certs[core][s] =
    (u_q_sorted, u_c_sorted, v_q_sorted, cand_sorted_uv, query_sorted_uv)
    for the exactness certificate + fallback."""
    import ml_dtypes
    bf = ml_dtypes.bfloat16
    pred = np.asarray(pred, dtype=np.float32)
    gt = np.asarray(gt, dtype=np.float32)
    gt_valid = np.asarray(gt_valid, dtype=np.float32)
    W = (nwin - P) // 2
    rhs_cols = NCH * nwin
    onesP = np.ones(PNUM, bf)
    in_maps = []
    certs = []
    # per-chunk candidate rank windows (shared across sides): ranks clipped
    base = np.arange(NCH)[:, None] * P - W + np.arange(nwin)[None, :]
    widx = np.clip(base, 0, PNUM - 1)          # [NCH, nwin]
    for core in range(NCORES):
        pemat = np.zeros((NSIDES, KS, PNUM + rhs_cols), bf)
        vwin = np.zeros((NSIDES, 3, rhs_cols), bf)
        vqneg = np.empty((NSIDES, P, NCH), np.float32)
        core_cert = []
        for i in range(BPC):
            b = core * BPC + i
            for side in range(2):
                s = i * 2 + side
                if side == 0:   # pred2gt: candidates pred, queries gt
                    cand, query = pred[b], gt[b]
                else:           # gt2pred: candidates gt_valid, queries pred
                    cand, query = gt_valid[b], pred[b]
                u_c = cand[:, 0] + cand[:, 1]
                v_c = cand[:, 0] - cand[:, 1]
                u_q = query[:, 0] + query[:, 1]
                v_q = query[:, 0] - query[:, 1]
                qord = np.argsort(u_q, kind="stable")
                cord = np.argsort(u_c, kind="stable")
                u_qs, v_qs = u_q[qord], v_q[qord]
                u_cs, v_cs = u_c[cord], v_c[cord]
                a0, a1, a2 = _split3_bf16(-u_qs)
                pemat[s, 0, :PNUM] = a0
                pemat[s, 1, :PNUM] = a1
                pemat[s, 2, :PNUM] = a2
                pemat[s, 3:6, :PNUM] = onesP
                ucw = u_cs[widx].reshape(-1)     # [NCH*nwin]
                vcw = v_cs[widx].reshape(-1)
                c0, c1, c2 = _split3_bf16(ucw)
                pemat[s, 0:3, PNUM:] = 1.0
                pemat[s, 3, PNUM:] = c0
                pemat[s, 4, PNUM:] = c1
                pemat[s, 5, PNUM:] = c2
                d0, d1, d2 = _split3_bf16(vcw)
                vwin[s, 0] = d0
                vwin[s, 1] = d1
                vwin[s, 2] = d2
                vqneg[s] = (-v_qs).reshape(NCH, P).T
                core_cert.append((u_qs, u_cs, v_qs, v_cs))
        in_maps.append({"pemat": pemat, "vwin": vwin, "vqneg": vqneg})
        certs.append(core_cert)
    return in_maps, certs


def _host_prep_win2(pred, gt, gt_valid, nwin):
    """Prep for _build_win2: one bf16 pemat per core with K=12 rows.
    Returns (in_maps, certs); certs identical to _host_prep_win."""
    import ml_dtypes
    bf = ml_dtypes.bfloat16
    pred = np.asarray(pred, dtype=np.float32)
    gt = np.asarray(gt, dtype=np.float32)
    gt_valid = np.asarray(gt_valid, dtype=np.float32)
    W = (nwin - P) // 2
    rhs_cols = NCH * 2 * nwin
    in_maps = []
    certs = []
    base = np.arange(NCH)[:, None] * P - W + np.arange(nwin)[None, :]
    widx = np.clip(base, 0, PNUM - 1)          # [NCH, nwin]
    for core in range(NCORES):
        pemat = np.zeros((NSIDES, 12, PNUM + rhs_cols), bf)
        core_cert = []
        for i in range(BPC):
            b = core * BPC + i
            for side in range(2):
                s = i * 2 + side
                if side == 0:
                    cand, query = pred[b], gt[b]
                else:
                    cand, query = gt_valid[b], pred[b]
                u_c = cand[:, 0] + cand[:, 1]
                v_c = cand[:, 0] - cand[:, 1]
                u_q = query[:, 0] + query[:, 1]
                v_q = query[:, 0] - query[:, 1]
                qord = np.argsort(u_q, kind="stable")
                cord = np.argsort(u_c, kind="stable")
                u_qs, v_qs = u_q[qord], v_q[qord]
                u_cs, v_cs = u_c[cord], v_c[cord]
                # lhsT [12, PNUM]: -u_q splits, ones, -v_q splits, ones
                a = _split3_bf16(-u_qs)
                e = _split3_bf16(-v_qs)
                for r in range(3):
                    pemat[s, r, :PNUM] = a[r]
                    pemat[s, 6 + r, :PNUM] = e[r]
                pemat[s, 3:6, :PNUM] = 1.0
                pemat[s, 9:12, :PNUM] = 1.0
                # rhs: per chunk, du cols then dv cols
                bspl = _split3_bf16(u_cs[widx])     # each [NCH, nwin]
                dspl = _split3_bf16(v_cs[widx])
                rhs = np.zeros((12, NCH, 2 * nwin), bf)
                rhs[0:3, :, :nwin] = 1.0
                for r in range(3):
                    rhs[3 + r, :, :nwin] = bspl[r]
                rhs[6:9, :, nwin:] = 1.0
                for r in range(3):
                    rhs[9 + r, :, nwin:] = dspl[r]
                pemat[s, :, PNUM:] = rhs.reshape(12, rhs_cols)
                core_cert.append((u_qs, u_cs, v_qs, v_cs))
        in_maps.append({"pemat": pemat})
        certs.append(core_cert)
    return in_maps, certs


def _certify_and_fix(mins_dev, certs, nwin):
    """mins_dev: [cores, NSIDES, P, NCH] device window-minima in sorted-query
    order (query rank r = c*P + p). Verify each against the u-gap to the
    nearest excluded candidate; recompute failures exactly. Returns
    (mins_fixed flat [cores, NSIDES, PNUM], n_fallback)."""
    W = (nwin - P) // 2
    out = np.empty((len(certs), NSIDES, PNUM), np.float64)
    n_fb = 0
    ranks = np.arange(PNUM)
    chunk = ranks // P
    lo_eff = np.maximum(chunk * P - W, 0)                 # [PNUM]
    hi_eff = np.minimum(chunk * P + (P - 1) + W, PNUM - 1)
    for ci, core_cert in enumerate(certs):
        for s, (u_qs, u_cs, v_qs, v_cs) in enumerate(core_cert):
            m = mins_dev[ci, s].T.reshape(-1).astype(np.float64)  # rank order
            gap_l = np.where(
                lo_eff > 0, u_qs - u_cs[np.maximum(lo_eff - 1, 0)], np.inf
            )
            gap_r = np.where(
                hi_eff < PNUM - 1, u_cs[np.minimum(hi_eff + 1, PNUM - 1)] - u_qs,
                np.inf,
            )
            bad = m > np.minimum(gap_l, gap_r)
            if bad.any():
                n_fb += int(bad.sum())
                uq, vq = u_qs[bad], v_qs[bad]
                du = np.abs(u_cs[None, :] - uq[:, None])
                dv = np.abs(v_cs[None, :] - vq[:, None])
                m[bad] = np.maximum(du, dv).min(axis=1)
            out[ci, s] = m
    return out, n_fb


def kernel(pred, gt, gt_valid, loss_type, _want_results=False):
    assert int(loss_type) == 1, f"only L1 supported, got {loss_type}"
    m, nwin = _mode()
    nc = _get_nc()
    if m == "win":
        in_maps, certs = _host_prep_win(pred, gt, gt_valid, nwin)
    elif m in ("win2", "seg"):
        in_maps, certs = _host_prep_win2(pred, gt, gt_valid, nwin)
    else:
        in_maps = _host_prep(pred, gt, gt_valid)
        certs = None
    res = run_bass_kernel_spmd(
        nc, in_maps, core_ids=list(range(NCORES)),
        trace=os.environ.get("DML_TRACE", "0") == "1",
    )
    mins = np.stack([res.results[c]["mins"] for c in range(NCORES)])
    # mins: [cores, NSIDES, P, NCH]; side = s % 2
    if m in ("win", "win2", "seg"):
        fixed, n_fb = _certify_and_fix(mins, certs, nwin)
        if os.environ.get("DML_VERBOSE"):
            print(f"[kernel] window fallbacks: {n_fb}")
        m_side = [fixed[:, side::2].mean() for side in range(2)]
    else:
        mins = mins.astype(np.float64)
        m_side = [mins[:, side::2].mean() for side in range(2)]
    out = np.float32((m_side[0] + m_side[1]) / 2.0)
    if _want_results:
        return out, res
    return out



# revision 8
# speedup vs baseline: 1.5803x; 1.5803x over previous
"""DML (Chamfer-style) L1 loss kernel for Trainium2, 8 NeuronCores.

Math: for each batch b:
  pred2gt_min[j] = min_i ||pred[b,i] - gt[b,j]||_1       (queries = gt)
  gt2pred_min[j] = min_i ||gt_valid[b,i] - pred[b,j]||_1 (queries = pred)
  out = (mean(pred2gt_min) + mean(gt2pred_min)) / 2

Device mapping (data-parallel over B across 8 cores; 4 batches x 2 sides
= 8 "batch-sides" per core). Rotate coords 45 deg on host (u = x+y,
v = x-y) so L1 dist = max(|du|, |dv|). Per 128-query x 2048-candidate tile:
  - TensorE: du[p, j] = u_c[j] - u_q[p] via a K=6 bf16 ones-matmul:
      u split 3-way into bf16 parts (u = a0+a1+a2 with ~2^-27 residual);
      lhsT rows [-a0,-a1,-a2, 1,1,1], rhs rows [1,1,1, b0,b1,b2].
      All products have a 1.0 factor so they are exact; PSUM accumulates
      in fp32 -> du exact to ~5e-8. bf16 matmul streams 1 column/cycle
      (fp32 would be 4x slower).
  - ScalarE: |dv| = Abs(v_c_rep - v_q[p]) via Abs activation with
      per-partition bias (v_c replicated across partitions by DMA).
  - VectorE: one fused custom-DVE op per tile:
      out = max(max(du, -du), |dv|);  accum_out = min-reduce(out)
      (registered per-NEFF via the custom DVE table mechanism).
  - Host: means in float64, final scalar.
"""
import os
import numpy as np

import concourse.bacc as bacc
import concourse.mybir as mybir
import concourse.tile as tile
from concourse.bass_utils import run_bass_kernel_spmd

F32 = mybir.dt.float32
BF16 = mybir.dt.bfloat16
B, PNUM, D = 32, 2048, 2
NCORES = 8
BPC = B // NCORES          # batches per core
NSIDES = 2 * BPC           # batch-sides per core
P = 128                    # SBUF partitions
NCH = PNUM // P            # query chunks per batch-side
KS = 6                     # matmul contraction: 3 bf16 splits x 2 operands
BIG = 3.0e38

_CACHED = {}


def _register_fused_op():
    """Per-NEFF custom DVE op:
        out = max(max(in0, -in0), in1);  accum_out = min(s0, min(out))
    i.e. a fused |du| + max + min-reduce (the TensorTensorReduce the
    stock ucode lacks, with the abs folded in).
    """
    import concourse.dve_ops as dve_ops
    name = "MIN_OF_ABSMAX_ANT"
    if "fused_op" in _CACHED:
        return _CACHED["fused_op"]
    for o in dve_ops.OPS:
        if o.name == name:
            _CACHED["fused_op"] = o
            return o
    from concourse.dve_spec import Spec, Src0, Src1, C0, Zero, maxx, minn, lower
    from concourse.dve_uop import DveOpSpec

    spec = Spec(body=maxx(maxx(Src0, Zero - Src0), Src1), accum=minn, accum_init=C0)
    row = max(dve_ops._SUB_OPCODE_FOR_NAME.values()) + 1
    assert row < 0x20, "no free custom-DVE opcode rows"
    dve_ops._SUB_OPCODE_FOR_NAME[name] = row
    shas = {}
    for ver in ("v3", "v4"):
        tmp = DveOpSpec(name=name, opcode=row, uops=lower(spec, ver=ver), rd1_en=True)
        shas[ver] = tmp.sha(ver)
    op = dve_ops.DveOp(name, spec, subdim=False, uops_sha=shas)
    dve_ops.OPS.append(op)
    dve_ops.CUSTOM_DVE_SPECS[name] = spec
    _CACHED["fused_op"] = op
    return op


def _register_segmin_op():
    """Hand-edited custom DVE op SEGMIN_ABSMAX1_ANT:
        streams in0 (du, PSUM) and in1 (|dv|, SBUF — pre-abs'ed by ACT;
        the DVE reads at most one PSUM stream) with 3D [P, S, N] APs;
        value = min over j<=k of max(|in0[p,s,j]|, in1[p,s,j]), with the
        running min RESET at each page (subdim) boundary.
    Lower a plain global-scan spec, then add a SUB_DIM_DONE step state
    that reseeds the scan feedback flop from CONST_0 (s0=BIG) while
    consuming the first element of the new page — a segmented min-reduce,
    one instruction per S chunks.  `out` is a stride-0-inner broadcast AP
    over the [P, S] mins tile: the hardware writes the running min every
    element to the same per-page address, so the LAST write (= the page
    minimum) survives — no separate extraction pass."""
    import copy
    import concourse.dve_ops as dve_ops
    from concourse.dve_spec import Spec, Src0, Src1, C0, Zero, maxx, lower, scan, AluOp
    from concourse.dve_uop import DveOpSpec, AluInp, Trigger

    name = "SEGMIN_ABSMAX1_ANT"
    if "segop" in _CACHED:
        return _CACHED["segop"]
    for o in dve_ops.OPS:
        if o.name == name:
            _CACHED["segop"] = o
            return o

    e = maxx(maxx(Src0, Zero - Src0), Src1)
    spec = Spec(body=scan(AluOp.MIN, e, init=C0))
    row = max(dve_ops._SUB_OPCODE_FOR_NAME.values()) + 1
    assert row < 0x20, "no free custom-DVE opcode rows"
    dve_ops._SUB_OPCODE_FOR_NAME[name] = row

    shas = {}
    for ver in ("v3", "v4"):
        uops = lower(spec, ver=ver)
        assert len(uops) == 2, f"expected [seed, steady], got {len(uops)}"
        seed, steady = uops
        scan_blk = None
        for bi, blk in enumerate(steady.datapath_config):
            if blk.op == AluOp.MIN and (
                blk.alu_src0 == AluInp.CURR_ALU_OUT
                or blk.alu_src1 == AluInp.CURR_ALU_OUT
            ):
                scan_blk = bi
                break
        assert scan_blk is not None, "scan combine block not found"
        const_inp = seed.datapath_config[scan_blk].alu_src0
        assert AluInp.PREV_DELAY_0 <= const_inp <= AluInp.PREV_DELAY_0 + 5
        steady.trigger = (Trigger.SRC_TENSOR_DONE, Trigger.SUB_DIM_DONE, Trigger.NONE)
        steady.next_uop = (0, 2, 0)
        step = copy.deepcopy(steady)
        step.trigger = (Trigger.SRC_TENSOR_DONE, Trigger.SUB_DIM_DONE, Trigger.COUNT)
        step.next_uop = (0, 2, 1)
        step.repeat_count = 1
        blk = step.datapath_config[scan_blk]
        if blk.alu_src0 == AluInp.CURR_ALU_OUT:
            blk.alu_src0 = const_inp
        else:
            blk.alu_src1 = const_inp
        edited = DveOpSpec(name=name, opcode=row, uops=[seed, steady, step],
                           rd1_en=True)
        edited.validate(ver)
        shas[ver] = edited.sha(ver)
        dve_ops._COMPILE_CACHE[(name, ver)] = edited

    op = dve_ops.DveOp(name, spec, subdim=True, uops_sha=shas)
    dve_ops.OPS.append(op)
    dve_ops.CUSTOM_DVE_SPECS[name] = spec
    _CACHED["segop"] = op
    return op


def _build_seg(nwin: int, cpb: int = 4, repeat: int = 1):
    """Segmented-scan kernel: per cpb-chunk block, cpb K=12 matmuls fill one
    PSUM tile (one 512-f32 bank slot per chunk: du at [0,nwin), dv at
    [nwin,2nwin)); ONE batched ACT Abs moves dv→SBUF (the DVE reads at most
    one PSUM stream); ONE segmented-min DVE instruction reduces all cpb
    chunks, writing the page minima straight into the mins tile via a
    stride-0 out AP.  Per-instruction fixed costs amortize over cpb chunks."""
    assert NCH % cpb == 0 and 2 * nwin <= 512
    slot = 256 if 2 * nwin <= 256 else 512
    nbufs = max(2, 4096 // (cpb * slot))  # PSUM pipeline depth (8 banks total)
    nc = bacc.Bacc("TRN2", target_bir_lowering=False)
    rhs_cols = NCH * 2 * nwin
    pemat = nc.dram_tensor(
        "pemat", [NSIDES, 12, PNUM + rhs_cols], BF16, kind="ExternalInput"
    )
    outm = nc.dram_tensor("mins", [NSIDES, P, NCH], F32, kind="ExternalOutput")
    segop = _register_segmin_op()

    with tile.TileContext(nc) as tc:
        with (
            tc.tile_pool(name="inp", bufs=2) as inp,
            tc.tile_pool(name="work", bufs=4) as work,
            tc.tile_pool(name="outp", bufs=2) as outp,
            tc.tile_pool(name="ps", bufs=nbufs, space="PSUM") as ps,
        ):
            for rep in range(repeat):
                for s in range(NSIDES):
                    pm = inp.tile([12, PNUM + rhs_cols], BF16, tag="pm")
                    nc.gpsimd.dma_start(out=pm[:], in_=pemat[s])
                    mq = outp.tile([P, NCH], F32, tag="mq")
                    for b in range(NCH // cpb):
                        duv = ps.tile([P, cpb, slot], F32, tag="duv")
                        for j in range(cpb):
                            c = b * cpb + j
                            nc.tensor.matmul(
                                duv[:, j, 0:2 * nwin],
                                pm[:, c * P:(c + 1) * P],
                                pm[:, PNUM + c * 2 * nwin:PNUM + (c + 1) * 2 * nwin],
                                start=True,
                                stop=True,
                            )
                        dva = work.tile([P, cpb, nwin], F32, tag="dva")
                        nc.scalar.activation(
                            out=dva[:],
                            in_=duv[:, :, nwin:2 * nwin],
                            func=mybir.ActivationFunctionType.Abs,
                            bias=0.0,
                            scale=1.0,
                        )
                        mq_bc = mq[:, b * cpb:(b + 1) * cpb].rearrange(
                            "p (s n) -> p s n", n=1
                        ).broadcast_to([P, cpb, nwin])
                        nc.vector._custom_dve(
                            segop, out=mq_bc, in0=duv[:, :, 0:nwin],
                            in1=dva[:], s0=BIG,
                        )
                    nc.sync.dma_start(out=outm[s], in_=mq[:])
    nc.compile()
    return nc


def _build_win(nwin: int, repeat: int = 1):
    """Windowed kernel: queries and candidates sorted by u on host; query
    chunk c scans only the candidate-rank window [128c-W, 128c+127+W]
    (clipped; fixed width nwin). Exactness is certified on the host.

    Inputs per core:
      pemat [NSIDES, 6, PNUM + NCH*nwin] bf16 - lhsT query 3-splits then
            per-chunk candidate-u window 3-splits
      vwin  [NSIDES, 3, NCH*nwin] bf16 - per-chunk candidate-v 3-splits
            (broadcast to 128 partitions via a K=3 ones-matmul)
      vqneg [NSIDES, P, NCH] f32 - ACT bias (-v_q, sorted order)
    Output: mins [NSIDES, P, NCH] f32 (sorted query order).
    """
    nc = bacc.Bacc("TRN2", target_bir_lowering=False)
    rhs_cols = NCH * nwin
    pemat = nc.dram_tensor(
        "pemat", [NSIDES, KS, PNUM + rhs_cols], BF16, kind="ExternalInput"
    )
    vwin = nc.dram_tensor("vwin", [NSIDES, 3, rhs_cols], BF16, kind="ExternalInput")
    vqneg = nc.dram_tensor("vqneg", [NSIDES, P, NCH], F32, kind="ExternalInput")
    outm = nc.dram_tensor("mins", [NSIDES, P, NCH], F32, kind="ExternalOutput")
    fop = _register_fused_op()

    with tile.TileContext(nc) as tc:
        with (
            tc.tile_pool(name="ones", bufs=1) as onep,
            tc.tile_pool(name="inp", bufs=2) as inp,
            tc.tile_pool(name="work", bufs=6) as work,
            tc.tile_pool(name="outp", bufs=2) as outp,
            tc.tile_pool(name="ps", bufs=4, space="PSUM") as ps,
            tc.tile_pool(name="ps2", bufs=4, space="PSUM") as ps2,
        ):
            ones3 = onep.tile([3, P], BF16)
            nc.vector.memset(ones3[:], 1.0)
            for rep in range(repeat):
                for s in range(NSIDES):
                    pm = inp.tile([KS, PNUM + rhs_cols], BF16, tag="pm")
                    nc.gpsimd.dma_start(out=pm[:], in_=pemat[s])
                    vw = inp.tile([3, rhs_cols], BF16, tag="vw")
                    nc.gpsimd.dma_start(out=vw[:], in_=vwin[s])
                    vq = inp.tile([P, NCH], F32, tag="vq")
                    nc.gpsimd.dma_start(out=vq[:], in_=vqneg[s])
                    mq = outp.tile([P, NCH], F32, tag="mq")
                    for c in range(NCH):
                        du = ps.tile([P, nwin], F32, tag="du")
                        nc.tensor.matmul(
                            du[:],
                            pm[:, c * P:(c + 1) * P],
                            pm[:, PNUM + c * nwin:PNUM + (c + 1) * nwin],
                            start=True,
                            stop=True,
                        )
                        vbc = ps2.tile([P, nwin], F32, tag="vbc")
                        nc.tensor.matmul(
                            vbc[:],
                            ones3[:],
                            vw[:, c * nwin:(c + 1) * nwin],
                            start=True,
                            stop=True,
                        )
                        dva = work.tile([P, nwin], F32, tag="dva")
                        nc.scalar.activation(
                            out=dva[:],
                            in_=vbc[:],
                            func=mybir.ActivationFunctionType.Abs,
                            bias=vq[:, c:c + 1],
                            scale=1.0,
                        )
                        dmx = work.tile([P, nwin], F32, tag="dmx")
                        nc.vector._custom_dve(
                            fop, out=dmx[:], in0=du[:], in1=dva[:],
                            s0=BIG, accum_out=mq[:, c:c + 1],
                        )
                    nc.sync.dma_start(out=outm[s], in_=mq[:])
    nc.compile()
    return nc


def _build_win2(nwin: int, repeat: int = 1):
    """Like _build_win but one K=12 matmul per chunk computes both du and
    dv (signed) into one PSUM tile [P, 2*nwin]:
      lhsT rows: [-u_q splits(3), ones(3), -v_q splits(3), ones(3)]
      rhs du-cols: [1,1,1, b0,b1,b2, 0...]; dv-cols: [0..., 1,1,1, d0,d1,d2]
    ACT: |dv| = Abs(dv_psum) -> SBUF.  DVE: fused min-of-absmax.
    Inputs per core: pemat [NSIDES, 12, PNUM + NCH*2*nwin] bf16 only.
    """
    K12 = 12
    nc = bacc.Bacc("TRN2", target_bir_lowering=False)
    rhs_cols = NCH * 2 * nwin
    pemat = nc.dram_tensor(
        "pemat", [NSIDES, K12, PNUM + rhs_cols], BF16, kind="ExternalInput"
    )
    outm = nc.dram_tensor("mins", [NSIDES, P, NCH], F32, kind="ExternalOutput")
    fop = _register_fused_op()

    with tile.TileContext(nc) as tc:
        with (
            tc.tile_pool(name="inp", bufs=2) as inp,
            tc.tile_pool(name="work", bufs=6) as work,
            tc.tile_pool(name="outp", bufs=2) as outp,
            tc.tile_pool(name="ps", bufs=4, space="PSUM") as ps,
        ):
            for rep in range(repeat):
                for s in range(NSIDES):
                    pm = inp.tile([K12, PNUM + rhs_cols], BF16, tag="pm")
                    nc.gpsimd.dma_start(out=pm[:], in_=pemat[s])
                    mq = outp.tile([P, NCH], F32, tag="mq")
                    for c in range(NCH):
                        duv = ps.tile([P, 2 * nwin], F32, tag="duv")
                        nc.tensor.matmul(
                            duv[:],
                            pm[:, c * P:(c + 1) * P],
                            pm[:, PNUM + c * 2 * nwin:PNUM + (c + 1) * 2 * nwin],
                            start=True,
                            stop=True,
                        )
                        dva = work.tile([P, nwin], F32, tag="dva")
                        nc.scalar.activation(
                            out=dva[:],
                            in_=duv[:, nwin:2 * nwin],
                            func=mybir.ActivationFunctionType.Abs,
                            bias=0.0,
                            scale=1.0,
                        )
                        dmx = work.tile([P, nwin], F32, tag="dmx")
                        nc.vector._custom_dve(
                            fop, out=dmx[:], in0=duv[:, 0:nwin], in1=dva[:],
                            s0=BIG, accum_out=mq[:, c:c + 1],
                        )
                    nc.sync.dma_start(out=outm[s], in_=mq[:])
    nc.compile()
    return nc


def _build(repeat: int = 1):
    nc = bacc.Bacc("TRN2", target_bir_lowering=False)
    pemat = nc.dram_tensor("pemat", [NSIDES, KS, 2 * PNUM], BF16, kind="ExternalInput")
    vcand = nc.dram_tensor("vcand", [NSIDES, PNUM], F32, kind="ExternalInput")
    vqneg = nc.dram_tensor("vqneg", [NSIDES, P, NCH], F32, kind="ExternalInput")
    outm = nc.dram_tensor("mins", [NSIDES, P, NCH], F32, kind="ExternalOutput")
    fop = _register_fused_op()

    with tile.TileContext(nc) as tc:
        with (
            tc.tile_pool(name="inp", bufs=2) as inp,
            tc.tile_pool(name="work", bufs=3) as work,
            tc.tile_pool(name="outp", bufs=2) as outp,
            tc.tile_pool(name="ps", bufs=2, space="PSUM") as ps,
        ):
          for rep in range(repeat):
            for s in range(NSIDES):
                pm = inp.tile([KS, 2 * PNUM], BF16, tag="pm")
                nc.gpsimd.dma_start(out=pm[:], in_=pemat[s])
                vr = inp.tile([P, PNUM], F32, tag="vr")
                nc.gpsimd.dma_start(
                    out=vr[:], in_=vcand[s][None, :].broadcast_to([P, PNUM])
                )
                vq = inp.tile([P, NCH], F32, tag="vq")
                nc.gpsimd.dma_start(out=vq[:], in_=vqneg[s])
                mq = outp.tile([P, NCH], F32, tag="mq")
                for c in range(NCH):
                    du = ps.tile([P, PNUM], F32, tag="du")
                    for n in range(4):
                        nc.tensor.matmul(
                            du[:, n * 512:(n + 1) * 512],
                            pm[:, c * P:(c + 1) * P],
                            pm[:, PNUM + n * 512:PNUM + (n + 1) * 512],
                            start=True,
                            stop=True,
                        )
                    dva = work.tile([P, PNUM], F32, tag="dva")
                    nc.scalar.activation(
                        out=dva[:],
                        in_=vr[:],
                        func=mybir.ActivationFunctionType.Abs,
                        bias=vq[:, c:c + 1],
                        scale=1.0,
                    )
                    dmx = work.tile([P, PNUM], F32, tag="dmx")
                    nc.vector._custom_dve(
                        fop, out=dmx[:], in0=du[:], in1=dva[:],
                        s0=BIG, accum_out=mq[:, c:c + 1],
                    )
                nc.sync.dma_start(out=outm[s], in_=mq[:])
    nc.compile()
    return nc


def _mode():
    """(mode, nwin): mode 'seg' (default), 'win2', 'win', or 'full'."""
    m = os.environ.get("DML_MODE", "seg")
    nwin = int(os.environ.get("DML_NWIN", "192"))
    return m, nwin


def _get_nc(repeat: int = 1):
    m, nwin = _mode()
    key = ("nc", m, nwin, repeat)
    if key not in _CACHED:
        if m == "seg":
            cpb = int(os.environ.get("DML_CPB", "4"))
            _CACHED[key] = _build_seg(nwin, cpb, repeat)
        else:
            builder = {"win": _build_win, "win2": _build_win2}.get(m)
            _CACHED[key] = builder(nwin, repeat) if builder else _build(repeat)
    return _CACHED[key]


def _split3_bf16(x):
    """3-way bf16 split: x ~ s0+s1+s2 with ~2^-27 relative residual."""
    import ml_dtypes
    bf = ml_dtypes.bfloat16
    x = x.astype(np.float32)
    s0 = x.astype(bf)
    r1 = x - s0.astype(np.float32)
    s1 = r1.astype(bf)
    r2 = r1 - s1.astype(np.float32)
    s2 = r2.astype(bf)
    return s0, s1, s2


def _host_prep(pred, gt, gt_valid):
    import ml_dtypes
    bf = ml_dtypes.bfloat16
    pred = np.asarray(pred, dtype=np.float32)
    gt = np.asarray(gt, dtype=np.float32)
    gt_valid = np.asarray(gt_valid, dtype=np.float32)
    ones = np.ones(PNUM, bf)
    in_maps = []
    for core in range(NCORES):
        pemat = np.zeros((NSIDES, KS, 2 * PNUM), bf)
        vcand = np.empty((NSIDES, PNUM), np.float32)
        vqneg = np.empty((NSIDES, P, NCH), np.float32)
        for i in range(BPC):
            b = core * BPC + i
            for side in range(2):
                s = i * 2 + side
                if side == 0:   # pred2gt: candidates pred, queries gt
                    cand, query = pred[b], gt[b]
                else:           # gt2pred: candidates gt_valid, queries pred
                    cand, query = gt_valid[b], pred[b]
                u_c = cand[:, 0] + cand[:, 1]
                v_c = cand[:, 0] - cand[:, 1]
                u_q = query[:, 0] + query[:, 1]
                v_q = query[:, 0] - query[:, 1]
                a0, a1, a2 = _split3_bf16(-u_q)
                b0, b1, b2 = _split3_bf16(u_c)
                # lhsT half (queries): rows [-a0,-a1,-a2, 1,1,1]
                pemat[s, 0, :PNUM] = a0
                pemat[s, 1, :PNUM] = a1
                pemat[s, 2, :PNUM] = a2
                pemat[s, 3, :PNUM] = ones
                pemat[s, 4, :PNUM] = ones
                pemat[s, 5, :PNUM] = ones
                # rhs half (candidates): rows [1,1,1, b0,b1,b2]
                pemat[s, 0, PNUM:] = ones
                pemat[s, 1, PNUM:] = ones
                pemat[s, 2, PNUM:] = ones
                pemat[s, 3, PNUM:] = b0
                pemat[s, 4, PNUM:] = b1
                pemat[s, 5, PNUM:] = b2
                vcand[s] = v_c
                vqneg[s] = (-v_q).reshape(NCH, P).T
        in_maps.append({"pemat": pemat, "vcand": vcand, "vqneg": vqneg})
    return in_maps


def _host_prep_win(pred, gt, gt_valid, nwin):
    """Sorted-window prep. Returns (in_maps, certs) where certs[core][s] =
    (u_q_sorted, u_c_sorted, v_q_sorted, cand_sorted_uv, query_sorted_uv)
    for the exactness certificate + fallback."""
    import ml_dtypes
    bf = ml_dtypes.bfloat16
    pred = np.asarray(pred, dtype=np.float32)
    gt = np.asarray(gt, dtype=np.float32)
    gt_valid = np.asarray(gt_valid, dtype=np.float32)
    W = (nwin - P) // 2
    rhs_cols = NCH * nwin
    onesP = np.ones(PNUM, bf)
    in_maps = []
    certs = []
    # per-chunk candidate rank windows (shared across sides): ranks clipped
    base = np.arange(NCH)[:, None] * P - W + np.arange(nwin)[None, :]
    widx = np.clip(base, 0, PNUM - 1)          # [NCH, nwin]
    for core in range(NCORES):
        pemat = np.zeros((NSIDES, KS, PNUM + rhs_cols), bf)
        vwin = np.zeros((NSIDES, 3, rhs_cols), bf)
        vqneg = np.empty((NSIDES, P, NCH), np.float32)
        core_cert = []
        for i in range(BPC):
            b = core * BPC + i
            for side in range(2):
                s = i * 2 + side
                if side == 0:   # pred2gt: candidates pred, queries gt
                    cand, query = pred[b], gt[b]
                else:           # gt2pred: candidates gt_valid, queries pred
                    cand, query = gt_valid[b], pred[b]
                u_c = cand[:, 0] + cand[:, 1]
                v_c = cand[:, 0] - cand[:, 1]
                u_q = query[:, 0] + query[:, 1]
                v_q = query[:, 0] - query[:, 1]
                qord = np.argsort(u_q, kind="stable")
                cord = np.argsort(u_c, kind="stable")
                u_qs, v_qs = u_q[qord], v_q[qord]
                u_cs, v_cs = u_c[cord], v_c[cord]
                a0, a1, a2 = _split3_bf16(-u_qs)
                pemat[s, 0, :PNUM] = a0
                pemat[s, 1, :PNUM] = a1
                pemat[s, 2, :PNUM] = a2
                pemat[s, 3:6, :PNUM] = onesP
                ucw = u_cs[widx].reshape(-1)     # [NCH*nwin]
                vcw = v_cs[widx].reshape(-1)
                c0, c1, c2 = _split3_bf16(ucw)
                pemat[s, 0:3, PNUM:] = 1.0
                pemat[s, 3, PNUM:] = c0
                pemat[s, 4, PNUM:] = c1
                pemat[s, 5, PNUM:] = c2
                d0, d1, d2 = _split3_bf16(vcw)
                vwin[s, 0] = d0
                vwin[s, 1] = d1
                vwin[s, 2] = d2
                vqneg[s] = (-v_qs).reshape(NCH, P).T
                core_cert.append((u_qs, u_cs, v_qs, v_cs))
        in_maps.append({"pemat": pemat, "vwin": vwin, "vqneg": vqneg})
        certs.append(core_cert)
    return in_maps, certs


def _host_prep_win2(pred, gt, gt_valid, nwin):
    """Prep for _build_win2: one bf16 pemat per core with K=12 rows.
    Returns (in_maps, certs); certs identical to _host_prep_win."""
    import ml_dtypes
    bf = ml_dtypes.bfloat16
    pred = np.asarray(pred, dtype=np.float32)
    gt = np.asarray(gt, dtype=np.float32)
    gt_valid = np.asarray(gt_valid, dtype=np.float32)
    W = (nwin - P) // 2
    rhs_cols = NCH * 2 * nwin
    in_maps = []
    certs = []
    base = np.arange(NCH)[:, None] * P - W + np.arange(nwin)[None, :]
    widx = np.clip(base, 0, PNUM - 1)          # [NCH, nwin]
    for core in range(NCORES):
        pemat = np.zeros((NSIDES, 12, PNUM + rhs_cols), bf)
        core_cert = []
        for i in range(BPC):
            b = core * BPC + i
            for side in range(2):
                s = i * 2 + side
                if side == 0:
                    cand, query = pred[b], gt[b]
                else:
                    cand, query = gt_valid[b], pred[b]
                u_c = cand[:, 0] + cand[:, 1]
                v_c = cand[:, 0] - cand[:, 1]
                u_q = query[:, 0] + query[:, 1]
                v_q = query[:, 0] - query[:, 1]
                qord = np.argsort(u_q, kind="stable")
                cord = np.argsort(u_c, kind="stable")
                u_qs, v_qs = u_q[qord], v_q[qord]
                u_cs, v_cs = u_c[cord], v_c[cord]
                # lhsT [12, PNUM]: -u_q splits, ones, -v_q splits, ones
                a = _split3_bf16(-u_qs)
                e = _split3_bf16(-v_qs)
                for r in range(3):
                    pemat[s, r, :PNUM] = a[r]
                    pemat[s, 6 + r, :PNUM] = e[r]
                pemat[s, 3:6, :PNUM] = 1.0
                pemat[s, 9:12, :PNUM] = 1.0
                # rhs: per chunk, du cols then dv cols
                bspl = _split3_bf16(u_cs[widx])     # each [NCH, nwin]
                dspl = _split3_bf16(v_cs[widx])
                rhs = np.zeros((12, NCH, 2 * nwin), bf)
                rhs[0:3, :, :nwin] = 1.0
                for r in range(3):
                    rhs[3 + r, :, :nwin] = bspl[r]
                rhs[6:9, :, nwin:] = 1.0
                for r in range(3):
                    rhs[9 + r, :, nwin:] = dspl[r]
                pemat[s, :, PNUM:] = rhs.reshape(12, rhs_cols)
                core_cert.append((u_qs, u_cs, v_qs, v_cs))
        in_maps.append({"pemat": pemat})
        certs.append(core_cert)
    return in_maps, certs


def _certify_and_fix(mins_dev, certs, nwin):
    """mins_dev: [cores, NSIDES, P, NCH] device window-minima in sorted-query
    order (query rank r = c*P + p). Verify each against the u-gap to the
    nearest excluded candidate; recompute failures exactly. Returns
    (mins_fixed flat [cores, NSIDES, PNUM], n_fallback)."""
    W = (nwin - P) // 2
    out = np.empty((len(certs), NSIDES, PNUM), np.float64)
    n_fb = 0
    ranks = np.arange(PNUM)
    chunk = ranks // P
    lo_eff = np.maximum(chunk * P - W, 0)                 # [PNUM]
    hi_eff = np.minimum(chunk * P + (P - 1) + W, PNUM - 1)
    for ci, core_cert in enumerate(certs):
        for s, (u_qs, u_cs, v_qs, v_cs) in enumerate(core_cert):
            m = mins_dev[ci, s].T.reshape(-1).astype(np.float64)  # rank order
            gap_l = np.where(
                lo_eff > 0, u_qs - u_cs[np.maximum(lo_eff - 1, 0)], np.inf
            )
            gap_r = np.where(
                hi_eff < PNUM - 1, u_cs[np.minimum(hi_eff + 1, PNUM - 1)] - u_qs,
                np.inf,
            )
            bad = m > np.minimum(gap_l, gap_r)
            if bad.any():
                n_fb += int(bad.sum())
                uq, vq = u_qs[bad], v_qs[bad]
                du = np.abs(u_cs[None, :] - uq[:, None])
                dv = np.abs(v_cs[None, :] - vq[:, None])
                m[bad] = np.maximum(du, dv).min(axis=1)
            out[ci, s] = m
    return out, n_fb


def kernel(pred, gt, gt_valid, loss_type, _want_results=False):
    assert int(loss_type) == 1, f"only L1 supported, got {loss_type}"
    m, nwin = _mode()
    nc = _get_nc()
    if m == "win":
        in_maps, certs = _host_prep_win(pred, gt, gt_valid, nwin)
    elif m in ("win2", "seg"):
        in_maps, certs = _host_prep_win2(pred, gt, gt_valid, nwin)
    else:
        in_maps = _host_prep(pred, gt, gt_valid)
        certs = None
    res = run_bass_kernel_spmd(
        nc, in_maps, core_ids=list(range(NCORES)),
        trace=os.environ.get("DML_TRACE", "0") == "1",
    )
    mins = np.stack([res.results[c]["mins"] for c in range(NCORES)])
    # mins: [cores, NSIDES, P, NCH]; side = s % 2
    if m in ("win", "win2", "seg"):
        fixed, n_fb = _certify_and_fix(mins, certs, nwin)
        if os.environ.get("DML_VERBOSE"):
            print(f"[kernel] window fallbacks: {n_fb}")
        m_side = [fixed[:, side::2].mean() for side in range(2)]
    else:
        mins = mins.astype(np.float64)
        m_side = [mins[:, side::2].mean() for side in range(2)]
    out = np.float32((m_side[0] + m_side[1]) / 2.0)
    if _want_results:
        return out, res
    return out



# revision 9
# speedup vs baseline: 1.6401x; 1.0378x over previous
"""DML (Chamfer-style) L1 loss kernel for Trainium2, 8 NeuronCores.

Math: for each batch b:
  pred2gt_min[j] = min_i ||pred[b,i] - gt[b,j]||_1       (queries = gt)
  gt2pred_min[j] = min_i ||gt_valid[b,i] - pred[b,j]||_1 (queries = pred)
  out = (mean(pred2gt_min) + mean(gt2pred_min)) / 2

Device mapping (data-parallel over B across 8 cores; 4 batches x 2 sides
= 8 "batch-sides" per core). Rotate coords 45 deg on host (u = x+y,
v = x-y) so L1 dist = max(|du|, |dv|). Per 128-query x 2048-candidate tile:
  - TensorE: du[p, j] = u_c[j] - u_q[p] via a K=6 bf16 ones-matmul:
      u split 3-way into bf16 parts (u = a0+a1+a2 with ~2^-27 residual);
      lhsT rows [-a0,-a1,-a2, 1,1,1], rhs rows [1,1,1, b0,b1,b2].
      All products have a 1.0 factor so they are exact; PSUM accumulates
      in fp32 -> du exact to ~5e-8. bf16 matmul streams 1 column/cycle
      (fp32 would be 4x slower).
  - ScalarE: |dv| = Abs(v_c_rep - v_q[p]) via Abs activation with
      per-partition bias (v_c replicated across partitions by DMA).
  - VectorE: one fused custom-DVE op per tile:
      out = max(max(du, -du), |dv|);  accum_out = min-reduce(out)
      (registered per-NEFF via the custom DVE table mechanism).
  - Host: means in float64, final scalar.
"""
import os
import numpy as np

import concourse.bacc as bacc
import concourse.mybir as mybir
import concourse.tile as tile
from concourse.bass_utils import run_bass_kernel_spmd

F32 = mybir.dt.float32
BF16 = mybir.dt.bfloat16
B, PNUM, D = 32, 2048, 2
NCORES = 8
BPC = B // NCORES          # batches per core
NSIDES = 2 * BPC           # batch-sides per core
P = 128                    # SBUF partitions
NCH = PNUM // P            # query chunks per batch-side
KS = 6                     # matmul contraction: 3 bf16 splits x 2 operands
BIG = 3.0e38

_CACHED = {}


def _register_fused_op():
    """Per-NEFF custom DVE op:
        out = max(max(in0, -in0), in1);  accum_out = min(s0, min(out))
    i.e. a fused |du| + max + min-reduce (the TensorTensorReduce the
    stock ucode lacks, with the abs folded in).
    """
    import concourse.dve_ops as dve_ops
    name = "MIN_OF_ABSMAX_ANT"
    if "fused_op" in _CACHED:
        return _CACHED["fused_op"]
    for o in dve_ops.OPS:
        if o.name == name:
            _CACHED["fused_op"] = o
            return o
    from concourse.dve_spec import Spec, Src0, Src1, C0, Zero, maxx, minn, lower
    from concourse.dve_uop import DveOpSpec

    spec = Spec(body=maxx(maxx(Src0, Zero - Src0), Src1), accum=minn, accum_init=C0)
    row = max(dve_ops._SUB_OPCODE_FOR_NAME.values()) + 1
    assert row < 0x20, "no free custom-DVE opcode rows"
    dve_ops._SUB_OPCODE_FOR_NAME[name] = row
    shas = {}
    for ver in ("v3", "v4"):
        tmp = DveOpSpec(name=name, opcode=row, uops=lower(spec, ver=ver), rd1_en=True)
        shas[ver] = tmp.sha(ver)
    op = dve_ops.DveOp(name, spec, subdim=False, uops_sha=shas)
    dve_ops.OPS.append(op)
    dve_ops.CUSTOM_DVE_SPECS[name] = spec
    _CACHED["fused_op"] = op
    return op


def _register_segmin_op():
    """Hand-edited custom DVE op SEGMIN_ABSMAX1_ANT:
        streams in0 (du, PSUM) and in1 (|dv|, SBUF — pre-abs'ed by ACT;
        the DVE reads at most one PSUM stream) with 3D [P, S, N] APs;
        value = min over j<=k of max(|in0[p,s,j]|, in1[p,s,j]), with the
        running min RESET at each page (subdim) boundary.
    Lower a plain global-scan spec, then add a SUB_DIM_DONE step state
    that reseeds the scan feedback flop from CONST_0 (s0=BIG) while
    consuming the first element of the new page — a segmented min-reduce,
    one instruction per S chunks.  `out` is a stride-0-inner broadcast AP
    over the [P, S] mins tile: the hardware writes the running min every
    element to the same per-page address, so the LAST write (= the page
    minimum) survives — no separate extraction pass."""
    import copy
    import concourse.dve_ops as dve_ops
    from concourse.dve_spec import Spec, Src0, Src1, C0, Zero, maxx, lower, scan, AluOp
    from concourse.dve_uop import DveOpSpec, AluInp, Trigger

    name = "SEGMIN_ABSMAX1_ANT"
    if "segop" in _CACHED:
        return _CACHED["segop"]
    for o in dve_ops.OPS:
        if o.name == name:
            _CACHED["segop"] = o
            return o

    e = maxx(maxx(Src0, Zero - Src0), Src1)
    spec = Spec(body=scan(AluOp.MIN, e, init=C0))
    row = max(dve_ops._SUB_OPCODE_FOR_NAME.values()) + 1
    assert row < 0x20, "no free custom-DVE opcode rows"
    dve_ops._SUB_OPCODE_FOR_NAME[name] = row

    shas = {}
    for ver in ("v3", "v4"):
        uops = lower(spec, ver=ver)
        assert len(uops) == 2, f"expected [seed, steady], got {len(uops)}"
        seed, steady = uops
        scan_blk = None
        for bi, blk in enumerate(steady.datapath_config):
            if blk.op == AluOp.MIN and (
                blk.alu_src0 == AluInp.CURR_ALU_OUT
                or blk.alu_src1 == AluInp.CURR_ALU_OUT
            ):
                scan_blk = bi
                break
        assert scan_blk is not None, "scan combine block not found"
        const_inp = seed.datapath_config[scan_blk].alu_src0
        assert AluInp.PREV_DELAY_0 <= const_inp <= AluInp.PREV_DELAY_0 + 5
        steady.trigger = (Trigger.SRC_TENSOR_DONE, Trigger.SUB_DIM_DONE, Trigger.NONE)
        steady.next_uop = (0, 2, 0)
        step = copy.deepcopy(steady)
        step.trigger = (Trigger.SRC_TENSOR_DONE, Trigger.SUB_DIM_DONE, Trigger.COUNT)
        step.next_uop = (0, 2, 1)
        step.repeat_count = 1
        blk = step.datapath_config[scan_blk]
        if blk.alu_src0 == AluInp.CURR_ALU_OUT:
            blk.alu_src0 = const_inp
        else:
            blk.alu_src1 = const_inp
        edited = DveOpSpec(name=name, opcode=row, uops=[seed, steady, step],
                           rd1_en=True)
        edited.validate(ver)
        shas[ver] = edited.sha(ver)
        dve_ops._COMPILE_CACHE[(name, ver)] = edited

    op = dve_ops.DveOp(name, spec, subdim=True, uops_sha=shas)
    dve_ops.OPS.append(op)
    dve_ops.CUSTOM_DVE_SPECS[name] = spec
    _CACHED["segop"] = op
    return op


def _build_seg(nwin: int, cpb: int = 4, repeat: int = 1):
    """Segmented-scan kernel: per cpb-chunk block, cpb K=12 matmuls fill one
    PSUM tile (one 512-f32 bank slot per chunk: du at [0,nwin), dv at
    [nwin,2nwin)); ONE batched ACT Abs moves dv→SBUF (the DVE reads at most
    one PSUM stream); ONE segmented-min DVE instruction reduces all cpb
    chunks, writing the page minima straight into the mins tile via a
    stride-0 out AP.  Per-instruction fixed costs amortize over cpb chunks."""
    assert NCH % cpb == 0 and 2 * nwin <= 512
    slot = 256 if 2 * nwin <= 256 else 512
    nbufs = max(2, 4096 // (cpb * slot))  # PSUM pipeline depth (8 banks total)
    nc = bacc.Bacc("TRN2", target_bir_lowering=False)
    rhs_cols = NCH * 2 * nwin
    pemat = nc.dram_tensor(
        "pemat", [NSIDES, 12, PNUM + rhs_cols], BF16, kind="ExternalInput"
    )
    outm = nc.dram_tensor("mins", [NSIDES, P, NCH], F32, kind="ExternalOutput")
    segop = _register_segmin_op()

    with tile.TileContext(nc) as tc:
        with (
            tc.tile_pool(name="inp", bufs=2) as inp,
            tc.tile_pool(name="work", bufs=4) as work,
            tc.tile_pool(name="outp", bufs=2) as outp,
            tc.tile_pool(name="ps", bufs=nbufs, space="PSUM") as ps,
        ):
            for rep in range(repeat):
                for s in range(NSIDES):
                    pm = inp.tile([12, PNUM + rhs_cols], BF16, tag="pm")
                    if s == 0 and rep == 0:
                        # fill-latency path: SP HWDGE (fast start) in two
                        # pieces so block-0 matmuls unblock after piece one
                        cut = PNUM + rhs_cols // 4
                        nc.sync.dma_start(out=pm[:, :cut], in_=pemat[s][:, :cut])
                        nc.sync.dma_start(out=pm[:, cut:], in_=pemat[s][:, cut:])
                    else:
                        nc.gpsimd.dma_start(out=pm[:], in_=pemat[s])
                    mq = outp.tile([P, NCH], F32, tag="mq")
                    for b in range(NCH // cpb):
                        duv = ps.tile([P, cpb, slot], F32, tag="duv")
                        for j in range(cpb):
                            c = b * cpb + j
                            nc.tensor.matmul(
                                duv[:, j, 0:2 * nwin],
                                pm[:, c * P:(c + 1) * P],
                                pm[:, PNUM + c * 2 * nwin:PNUM + (c + 1) * 2 * nwin],
                                start=True,
                                stop=True,
                            )
                        dva = work.tile([P, cpb, nwin], F32, tag="dva")
                        nc.scalar.activation(
                            out=dva[:],
                            in_=duv[:, :, nwin:2 * nwin],
                            func=mybir.ActivationFunctionType.Abs,
                            bias=0.0,
                            scale=1.0,
                        )
                        mq_bc = mq[:, b * cpb:(b + 1) * cpb].rearrange(
                            "p (s n) -> p s n", n=1
                        ).broadcast_to([P, cpb, nwin])
                        nc.vector._custom_dve(
                            segop, out=mq_bc, in0=duv[:, :, 0:nwin],
                            in1=dva[:], s0=BIG,
                        )
                    nc.sync.dma_start(out=outm[s], in_=mq[:])
    nc.compile()
    return nc


def _build_win(nwin: int, repeat: int = 1):
    """Windowed kernel: queries and candidates sorted by u on host; query
    chunk c scans only the candidate-rank window [128c-W, 128c+127+W]
    (clipped; fixed width nwin). Exactness is certified on the host.

    Inputs per core:
      pemat [NSIDES, 6, PNUM + NCH*nwin] bf16 - lhsT query 3-splits then
            per-chunk candidate-u window 3-splits
      vwin  [NSIDES, 3, NCH*nwin] bf16 - per-chunk candidate-v 3-splits
            (broadcast to 128 partitions via a K=3 ones-matmul)
      vqneg [NSIDES, P, NCH] f32 - ACT bias (-v_q, sorted order)
    Output: mins [NSIDES, P, NCH] f32 (sorted query order).
    """
    nc = bacc.Bacc("TRN2", target_bir_lowering=False)
    rhs_cols = NCH * nwin
    pemat = nc.dram_tensor(
        "pemat", [NSIDES, KS, PNUM + rhs_cols], BF16, kind="ExternalInput"
    )
    vwin = nc.dram_tensor("vwin", [NSIDES, 3, rhs_cols], BF16, kind="ExternalInput")
    vqneg = nc.dram_tensor("vqneg", [NSIDES, P, NCH], F32, kind="ExternalInput")
    outm = nc.dram_tensor("mins", [NSIDES, P, NCH], F32, kind="ExternalOutput")
    fop = _register_fused_op()

    with tile.TileContext(nc) as tc:
        with (
            tc.tile_pool(name="ones", bufs=1) as onep,
            tc.tile_pool(name="inp", bufs=2) as inp,
            tc.tile_pool(name="work", bufs=6) as work,
            tc.tile_pool(name="outp", bufs=2) as outp,
            tc.tile_pool(name="ps", bufs=4, space="PSUM") as ps,
            tc.tile_pool(name="ps2", bufs=4, space="PSUM") as ps2,
        ):
            ones3 = onep.tile([3, P], BF16)
            nc.vector.memset(ones3[:], 1.0)
            for rep in range(repeat):
                for s in range(NSIDES):
                    pm = inp.tile([KS, PNUM + rhs_cols], BF16, tag="pm")
                    nc.gpsimd.dma_start(out=pm[:], in_=pemat[s])
                    vw = inp.tile([3, rhs_cols], BF16, tag="vw")
                    nc.gpsimd.dma_start(out=vw[:], in_=vwin[s])
                    vq = inp.tile([P, NCH], F32, tag="vq")
                    nc.gpsimd.dma_start(out=vq[:], in_=vqneg[s])
                    mq = outp.tile([P, NCH], F32, tag="mq")
                    for c in range(NCH):
                        du = ps.tile([P, nwin], F32, tag="du")
                        nc.tensor.matmul(
                            du[:],
                            pm[:, c * P:(c + 1) * P],
                            pm[:, PNUM + c * nwin:PNUM + (c + 1) * nwin],
                            start=True,
                            stop=True,
                        )
                        vbc = ps2.tile([P, nwin], F32, tag="vbc")
                        nc.tensor.matmul(
                            vbc[:],
                            ones3[:],
                            vw[:, c * nwin:(c + 1) * nwin],
                            start=True,
                            stop=True,
                        )
                        dva = work.tile([P, nwin], F32, tag="dva")
                        nc.scalar.activation(
                            out=dva[:],
                            in_=vbc[:],
                            func=mybir.ActivationFunctionType.Abs,
                            bias=vq[:, c:c + 1],
                            scale=1.0,
                        )
                        dmx = work.tile([P, nwin], F32, tag="dmx")
                        nc.vector._custom_dve(
                            fop, out=dmx[:], in0=du[:], in1=dva[:],
                            s0=BIG, accum_out=mq[:, c:c + 1],
                        )
                    nc.sync.dma_start(out=outm[s], in_=mq[:])
    nc.compile()
    return nc


def _build_win2(nwin: int, repeat: int = 1):
    """Like _build_win but one K=12 matmul per chunk computes both du and
    dv (signed) into one PSUM tile [P, 2*nwin]:
      lhsT rows: [-u_q splits(3), ones(3), -v_q splits(3), ones(3)]
      rhs du-cols: [1,1,1, b0,b1,b2, 0...]; dv-cols: [0..., 1,1,1, d0,d1,d2]
    ACT: |dv| = Abs(dv_psum) -> SBUF.  DVE: fused min-of-absmax.
    Inputs per core: pemat [NSIDES, 12, PNUM + NCH*2*nwin] bf16 only.
    """
    K12 = 12
    nc = bacc.Bacc("TRN2", target_bir_lowering=False)
    rhs_cols = NCH * 2 * nwin
    pemat = nc.dram_tensor(
        "pemat", [NSIDES, K12, PNUM + rhs_cols], BF16, kind="ExternalInput"
    )
    outm = nc.dram_tensor("mins", [NSIDES, P, NCH], F32, kind="ExternalOutput")
    fop = _register_fused_op()

    with tile.TileContext(nc) as tc:
        with (
            tc.tile_pool(name="inp", bufs=2) as inp,
            tc.tile_pool(name="work", bufs=6) as work,
            tc.tile_pool(name="outp", bufs=2) as outp,
            tc.tile_pool(name="ps", bufs=4, space="PSUM") as ps,
        ):
            for rep in range(repeat):
                for s in range(NSIDES):
                    pm = inp.tile([K12, PNUM + rhs_cols], BF16, tag="pm")
                    nc.gpsimd.dma_start(out=pm[:], in_=pemat[s])
                    mq = outp.tile([P, NCH], F32, tag="mq")
                    for c in range(NCH):
                        duv = ps.tile([P, 2 * nwin], F32, tag="duv")
                        nc.tensor.matmul(
                            duv[:],
                            pm[:, c * P:(c + 1) * P],
                            pm[:, PNUM + c * 2 * nwin:PNUM + (c + 1) * 2 * nwin],
                            start=True,
                            stop=True,
                        )
                        dva = work.tile([P, nwin], F32, tag="dva")
                        nc.scalar.activation(
                            out=dva[:],
                            in_=duv[:, nwin:2 * nwin],
                            func=mybir.ActivationFunctionType.Abs,
                            bias=0.0,
                            scale=1.0,
                        )
                        dmx = work.tile([P, nwin], F32, tag="dmx")
                        nc.vector._custom_dve(
                            fop, out=dmx[:], in0=duv[:, 0:nwin], in1=dva[:],
                            s0=BIG, accum_out=mq[:, c:c + 1],
                        )
                    nc.sync.dma_start(out=outm[s], in_=mq[:])
    nc.compile()
    return nc


def _build(repeat: int = 1):
    nc = bacc.Bacc("TRN2", target_bir_lowering=False)
    pemat = nc.dram_tensor("pemat", [NSIDES, KS, 2 * PNUM], BF16, kind="ExternalInput")
    vcand = nc.dram_tensor("vcand", [NSIDES, PNUM], F32, kind="ExternalInput")
    vqneg = nc.dram_tensor("vqneg", [NSIDES, P, NCH], F32, kind="ExternalInput")
    outm = nc.dram_tensor("mins", [NSIDES, P, NCH], F32, kind="ExternalOutput")
    fop = _register_fused_op()

    with tile.TileContext(nc) as tc:
        with (
            tc.tile_pool(name="inp", bufs=2) as inp,
            tc.tile_pool(name="work", bufs=3) as work,
            tc.tile_pool(name="outp", bufs=2) as outp,
            tc.tile_pool(name="ps", bufs=2, space="PSUM") as ps,
        ):
          for rep in range(repeat):
            for s in range(NSIDES):
                pm = inp.tile([KS, 2 * PNUM], BF16, tag="pm")
                nc.gpsimd.dma_start(out=pm[:], in_=pemat[s])
                vr = inp.tile([P, PNUM], F32, tag="vr")
                nc.gpsimd.dma_start(
                    out=vr[:], in_=vcand[s][None, :].broadcast_to([P, PNUM])
                )
                vq = inp.tile([P, NCH], F32, tag="vq")
                nc.gpsimd.dma_start(out=vq[:], in_=vqneg[s])
                mq = outp.tile([P, NCH], F32, tag="mq")
                for c in range(NCH):
                    du = ps.tile([P, PNUM], F32, tag="du")
                    for n in range(4):
                        nc.tensor.matmul(
                            du[:, n * 512:(n + 1) * 512],
                            pm[:, c * P:(c + 1) * P],
                            pm[:, PNUM + n * 512:PNUM + (n + 1) * 512],
                            start=True,
                            stop=True,
                        )
                    dva = work.tile([P, PNUM], F32, tag="dva")
                    nc.scalar.activation(
                        out=dva[:],
                        in_=vr[:],
                        func=mybir.ActivationFunctionType.Abs,
                        bias=vq[:, c:c + 1],
                        scale=1.0,
                    )
                    dmx = work.tile([P, PNUM], F32, tag="dmx")
                    nc.vector._custom_dve(
                        fop, out=dmx[:], in0=du[:], in1=dva[:],
                        s0=BIG, accum_out=mq[:, c:c + 1],
                    )
                nc.sync.dma_start(out=outm[s], in_=mq[:])
    nc.compile()
    return nc


def _mode():
    """(mode, nwin): mode 'seg' (default), 'win2', 'win', or 'full'."""
    m = os.environ.get("DML_MODE", "seg")
    nwin = int(os.environ.get("DML_NWIN", "192"))
    return m, nwin


def _get_nc(repeat: int = 1):
    m, nwin = _mode()
    key = ("nc", m, nwin, repeat)
    if key not in _CACHED:
        if m == "seg":
            cpb = int(os.environ.get("DML_CPB", "4"))
            _CACHED[key] = _build_seg(nwin, cpb, repeat)
        else:
            builder = {"win": _build_win, "win2": _build_win2}.get(m)
            _CACHED[key] = builder(nwin, repeat) if builder else _build(repeat)
    return _CACHED[key]


def _split3_bf16(x):
    """3-way bf16 split: x ~ s0+s1+s2 with ~2^-27 relative residual."""
    import ml_dtypes
    bf = ml_dtypes.bfloat16
    x = x.astype(np.float32)
    s0 = x.astype(bf)
    r1 = x - s0.astype(np.float32)
    s1 = r1.astype(bf)
    r2 = r1 - s1.astype(np.float32)
    s2 = r2.astype(bf)
    return s0, s1, s2


def _host_prep(pred, gt, gt_valid):
    import ml_dtypes
    bf = ml_dtypes.bfloat16
    pred = np.asarray(pred, dtype=np.float32)
    gt = np.asarray(gt, dtype=np.float32)
    gt_valid = np.asarray(gt_valid, dtype=np.float32)
    ones = np.ones(PNUM, bf)
    in_maps = []
    for core in range(NCORES):
        pemat = np.zeros((NSIDES, KS, 2 * PNUM), bf)
        vcand = np.empty((NSIDES, PNUM), np.float32)
        vqneg = np.empty((NSIDES, P, NCH), np.float32)
        for i in range(BPC):
            b = core * BPC + i
            for side in range(2):
                s = i * 2 + side
                if side == 0:   # pred2gt: candidates pred, queries gt
                    cand, query = pred[b], gt[b]
                else:           # gt2pred: candidates gt_valid, queries pred
                    cand, query = gt_valid[b], pred[b]
                u_c = cand[:, 0] + cand[:, 1]
                v_c = cand[:, 0] - cand[:, 1]
                u_q = query[:, 0] + query[:, 1]
                v_q = query[:, 0] - query[:, 1]
                a0, a1, a2 = _split3_bf16(-u_q)
                b0, b1, b2 = _split3_bf16(u_c)
                # lhsT half (queries): rows [-a0,-a1,-a2, 1,1,1]
                pemat[s, 0, :PNUM] = a0
                pemat[s, 1, :PNUM] = a1
                pemat[s, 2, :PNUM] = a2
                pemat[s, 3, :PNUM] = ones
                pemat[s, 4, :PNUM] = ones
                pemat[s, 5, :PNUM] = ones
                # rhs half (candidates): rows [1,1,1, b0,b1,b2]
                pemat[s, 0, PNUM:] = ones
                pemat[s, 1, PNUM:] = ones
                pemat[s, 2, PNUM:] = ones
                pemat[s, 3, PNUM:] = b0
                pemat[s, 4, PNUM:] = b1
                pemat[s, 5, PNUM:] = b2
                vcand[s] = v_c
                vqneg[s] = (-v_q).reshape(NCH, P).T
        in_maps.append({"pemat": pemat, "vcand": vcand, "vqneg": vqneg})
    return in_maps


def _host_prep_win(pred, gt, gt_valid, nwin):
    """Sorted-window prep. Returns (in_maps, certs) where certs[core][s] =
    (u_q_sorted, u_c_sorted, v_q_sorted, cand_sorted_uv, query_sorted_uv)
    for the exactness certificate + fallback."""
    import ml_dtypes
    bf = ml_dtypes.bfloat16
    pred = np.asarray(pred, dtype=np.float32)
    gt = np.asarray(gt, dtype=np.float32)
    gt_valid = np.asarray(gt_valid, dtype=np.float32)
    W = (nwin - P) // 2
    rhs_cols = NCH * nwin
    onesP = np.ones(PNUM, bf)
    in_maps = []
    certs = []
    # per-chunk candidate rank windows (shared across sides): ranks clipped
    base = np.arange(NCH)[:, None] * P - W + np.arange(nwin)[None, :]
    widx = np.clip(base, 0, PNUM - 1)          # [NCH, nwin]
    for core in range(NCORES):
        pemat = np.zeros((NSIDES, KS, PNUM + rhs_cols), bf)
        vwin = np.zeros((NSIDES, 3, rhs_cols), bf)
        vqneg = np.empty((NSIDES, P, NCH), np.float32)
        core_cert = []
        for i in range(BPC):
            b = core * BPC + i
            for side in range(2):
                s = i * 2 + side
                if side == 0:   # pred2gt: candidates pred, queries gt
                    cand, query = pred[b], gt[b]
                else:           # gt2pred: candidates gt_valid, queries pred
                    cand, query = gt_valid[b], pred[b]
                u_c = cand[:, 0] + cand[:, 1]
                v_c = cand[:, 0] - cand[:, 1]
                u_q = query[:, 0] + query[:, 1]
                v_q = query[:, 0] - query[:, 1]
                qord = np.argsort(u_q, kind="stable")
                cord = np.argsort(u_c, kind="stable")
                u_qs, v_qs = u_q[qord], v_q[qord]
                u_cs, v_cs = u_c[cord], v_c[cord]
                a0, a1, a2 = _split3_bf16(-u_qs)
                pemat[s, 0, :PNUM] = a0
                pemat[s, 1, :PNUM] = a1
                pemat[s, 2, :PNUM] = a2
                pemat[s, 3:6, :PNUM] = onesP
                ucw = u_cs[widx].reshape(-1)     # [NCH*nwin]
                vcw = v_cs[widx].reshape(-1)
                c0, c1, c2 = _split3_bf16(ucw)
                pemat[s, 0:3, PNUM:] = 1.0
                pemat[s, 3, PNUM:] = c0
                pemat[s, 4, PNUM:] = c1
                pemat[s, 5, PNUM:] = c2
                d0, d1, d2 = _split3_bf16(vcw)
                vwin[s, 0] = d0
                vwin[s, 1] = d1
                vwin[s, 2] = d2
                vqneg[s] = (-v_qs).reshape(NCH, P).T
                core_cert.append((u_qs, u_cs, v_qs, v_cs))
        in_maps.append({"pemat": pemat, "vwin": vwin, "vqneg": vqneg})
        certs.append(core_cert)
    return in_maps, certs


def _host_prep_win2(pred, gt, gt_valid, nwin):
    """Prep for _build_win2: one bf16 pemat per core with K=12 rows.
    Returns (in_maps, certs); certs identical to _host_prep_win."""
    import ml_dtypes
    bf = ml_dtypes.bfloat16
    pred = np.asarray(pred, dtype=np.float32)
    gt = np.asarray(gt, dtype=np.float32)
    gt_valid = np.asarray(gt_valid, dtype=np.float32)
    W = (nwin - P) // 2
    rhs_cols = NCH * 2 * nwin
    in_maps = []
    certs = []
    base = np.arange(NCH)[:, None] * P - W + np.arange(nwin)[None, :]
    widx = np.clip(base, 0, PNUM - 1)          # [NCH, nwin]
    for core in range(NCORES):
        pemat = np.zeros((NSIDES, 12, PNUM + rhs_cols), bf)
        core_cert = []
        for i in range(BPC):
            b = core * BPC + i
            for side in range(2):
                s = i * 2 + side
                if side == 0:
                    cand, query = pred[b], gt[b]
                else:
                    cand, query = gt_valid[b], pred[b]
                u_c = cand[:, 0] + cand[:, 1]
                v_c = cand[:, 0] - cand[:, 1]
                u_q = query[:, 0] + query[:, 1]
                v_q = query[:, 0] - query[:, 1]
                qord = np.argsort(u_q, kind="stable")
                cord = np.argsort(u_c, kind="stable")
                u_qs, v_qs = u_q[qord], v_q[qord]
                u_cs, v_cs = u_c[cord], v_c[cord]
                # lhsT [12, PNUM]: -u_q splits, ones, -v_q splits, ones
                a = _split3_bf16(-u_qs)
                e = _split3_bf16(-v_qs)
                for r in range(3):
                    pemat[s, r, :PNUM] = a[r]
                    pemat[s, 6 + r, :PNUM] = e[r]
                pemat[s, 3:6, :PNUM] = 1.0
                pemat[s, 9:12, :PNUM] = 1.0
                # rhs: per chunk, du cols then dv cols
                bspl = _split3_bf16(u_cs[widx])     # each [NCH, nwin]
                dspl = _split3_bf16(v_cs[widx])
                rhs = np.zeros((12, NCH, 2 * nwin), bf)
                rhs[0:3, :, :nwin] = 1.0
                for r in range(3):
                    rhs[3 + r, :, :nwin] = bspl[r]
                rhs[6:9, :, nwin:] = 1.0
                for r in range(3):
                    rhs[9 + r, :, nwin:] = dspl[r]
                pemat[s, :, PNUM:] = rhs.reshape(12, rhs_cols)
                core_cert.append((u_qs, u_cs, v_qs, v_cs))
        in_maps.append({"pemat": pemat})
        certs.append(core_cert)
    return in_maps, certs


def _certify_and_fix(mins_dev, certs, nwin):
    """mins_dev: [cores, NSIDES, P, NCH] device window-minima in sorted-query
    order (query rank r = c*P + p). Verify each against the u-gap to the
    nearest excluded candidate; recompute failures exactly. Returns
    (mins_fixed flat [cores, NSIDES, PNUM], n_fallback)."""
    W = (nwin - P) // 2
    out = np.empty((len(certs), NSIDES, PNUM), np.float64)
    n_fb = 0
    ranks = np.arange(PNUM)
    chunk = ranks // P
    lo_eff = np.maximum(chunk * P - W, 0)                 # [PNUM]
    hi_eff = np.minimum(chunk * P + (P - 1) + W, PNUM - 1)
    for ci, core_cert in enumerate(certs):
        for s, (u_qs, u_cs, v_qs, v_cs) in enumerate(core_cert):
            m = mins_dev[ci, s].T.reshape(-1).astype(np.float64)  # rank order
            gap_l = np.where(
                lo_eff > 0, u_qs - u_cs[np.maximum(lo_eff - 1, 0)], np.inf
            )
            gap_r = np.where(
                hi_eff < PNUM - 1, u_cs[np.minimum(hi_eff + 1, PNUM - 1)] - u_qs,
                np.inf,
            )
            bad = m > np.minimum(gap_l, gap_r)
            if bad.any():
                n_fb += int(bad.sum())
                uq, vq = u_qs[bad], v_qs[bad]
                du = np.abs(u_cs[None, :] - uq[:, None])
                dv = np.abs(v_cs[None, :] - vq[:, None])
                m[bad] = np.maximum(du, dv).min(axis=1)
            out[ci, s] = m
    return out, n_fb


def kernel(pred, gt, gt_valid, loss_type, _want_results=False):
    assert int(loss_type) == 1, f"only L1 supported, got {loss_type}"
    m, nwin = _mode()
    nc = _get_nc()
    if m == "win":
        in_maps, certs = _host_prep_win(pred, gt, gt_valid, nwin)
    elif m in ("win2", "seg"):
        in_maps, certs = _host_prep_win2(pred, gt, gt_valid, nwin)
    else:
        in_maps = _host_prep(pred, gt, gt_valid)
        certs = None
    res = run_bass_kernel_spmd(
        nc, in_maps, core_ids=list(range(NCORES)),
        trace=os.environ.get("DML_TRACE", "0") == "1",
    )
    mins = np.stack([res.results[c]["mins"] for c in range(NCORES)])
    # mins: [cores, NSIDES, P, NCH]; side = s % 2
    if m in ("win", "win2", "seg"):
        fixed, n_fb = _certify_and_fix(mins, certs, nwin)
        if os.environ.get("DML_VERBOSE"):
            print(f"[kernel] window fallbacks: {n_fb}")
        m_side = [fixed[:, side::2].mean() for side in range(2)]
    else:
        mins = mins.astype(np.float64)
        m_side = [mins[:, side::2].mean() for side in range(2)]
    out = np.float32((m_side[0] + m_side[1]) / 2.0)
    if _want_results:
        return out, res
    return out

